# revision 13
# baseline (speedup 1.0000x reference)
"""Trainium2 Bass kernel for nn_ChallengingGeometricLoss.

Computes loss = 0.1 * mean(exp(-0.1 * cdist(x, x)))  for x = embeddings
reshaped to [N=8192, d=512], plus total = 0.5 * loss.

Strategy (8 NeuronCores, SPMD, identical program per core):
  - Rows are grouped in 16 super-blocks of 512. Super-block B computes its
    [512, 4608] cyclic band of the pairwise matrix: columns
    [512*B, 512*B + 4608) mod 8192 (its own diagonal block plus the next
    8 blocks).  With T = sum over bands, D0 = the delta=0 (diagonal-block)
    tiles, D8 = the delta=8 tiles:  S_offdiag+diag = D0 + 2*T_mid + D8.
    True diagonal entries are masked to ~0 on device (big-identity
    subtraction in PSUM) and the exact +N is added on the host.
  - Core c owns super-blocks {2c, 2c+1}; all the columns it needs form a
    contiguous (mod N) window of 5120 rows, prepared host-side per core.
  - Per [128, 512] psum tile: a K=1 fp16 matmul broadcasts -a_j/2 into
    PSUM (start=True), then 4 K=128 fp16 matmuls accumulate x @ x.T.
    ACT computes dist = Sqrt(-2*psum + a_i) (bias = per-partition row
    norms) into a big fp16 SBUF buffer; after ALL sqrts (one activation
    table), a second ACT pass computes Exp(-0.1*dist) with accum_out
    (one table switch total), giving per-partition partial sums.
  - DVE reduces the accumulator columns, a ones-matmul reduces across
    partitions, and each core DMAs out two scalars [d_sum, t_sum].
    Host: S = sum_c (d_c + 2*t_c) + N;  loss = 0.1 * S / N^2.
"""

import numpy as np

import concourse.bass as bass
import concourse.mybir as mybir
import concourse.tile as tile
from concourse import bacc
from concourse.bass_utils import run_bass_kernel_spmd
from concourse.tile import add_dep_helper

# Problem constants (hardcoded per contract).
N = 8192
D = 512
NCORES = 8
P = 128
KC = D // P            # 4 k-chunks of 128
NBLK = 2               # local 512-row super-blocks per core
NSUB = 4               # 128-row sub-blocks per super-block
BAND = 4608            # band width per sub-block (9 x 512)
WIN = 5120             # per-core column window (2*512 + 4608)
GRP = 1536             # psum group: 3 banks = 3 x 512 cols
NG = BAND // GRP       # 3 psum groups per band
TPG = GRP // 512       # 3 x 512-col tiles per group
NU = NBLK * NSUB       # 8 sub-blocks per core
BIGVAL = 60000.0       # diagonal mask magnitude (exactly representable in fp16)

dt = mybir.dt
AF = mybir.ActivationFunctionType


def build_program(main_dtype=dt.float16):
    """Build the per-core Bass/Tile program (identical across cores)."""
    nc = bacc.Bacc("TRN2", num_devices=NCORES, debug=False)

    xtw_d = nc.dram_tensor("xtw", [KC, P, WIN], main_dtype, kind="ExternalInput")
    negah_d = nc.dram_tensor("negah", [1, WIN], dt.float16, kind="ExternalInput")
    arows_d = nc.dram_tensor("arows", [P, NU], dt.float32, kind="ExternalInput")
    ident_d = nc.dram_tensor("ident", [P, P], dt.float16, kind="ExternalInput")
    negbig_d = nc.dram_tensor("negbig", [P, P], dt.float16, kind="ExternalInput")
    ones1_d = nc.dram_tensor("ones1", [1, P], dt.float16, kind="ExternalInput")
    onescol_d = nc.dram_tensor("onescol", [P, 1], dt.float32, kind="ExternalInput")
    out_d = nc.dram_tensor("out2", [2, 1], dt.float32, kind="ExternalOutput")

    with tile.TileContext(nc) as tc:
        with (
            tc.tile_pool(name="big", bufs=1) as bigp,
            tc.tile_pool(name="small", bufs=1) as smallp,
            tc.tile_pool(name="psum", bufs=2, space="PSUM") as psump,
            tc.tile_pool(name="psum1", bufs=1, space="PSUM") as psump1,
        ):
            xtw = bigp.tile([P, KC, WIN], main_dtype, tag="xtw")
            dist = bigp.tile([P, NU * BAND], dt.float16, tag="dist")
            negah = smallp.tile([1, WIN], dt.float16, tag="negah")
            arows = smallp.tile([P, NU], dt.float32, tag="arows")
            ident = smallp.tile([P, P], dt.float16, tag="ident")
            negbig = smallp.tile([P, P], dt.float16, tag="negbig")
            ones1 = smallp.tile([1, P], dt.float16, tag="ones1")
            onescol = smallp.tile([P, 1], dt.float32, tag="onescol")
            acc = smallp.tile([P, 3 * NU], dt.float32, tag="acc")
            red2 = smallp.tile([P, 2], dt.float32, tag="red2")
            outsb = smallp.tile([2, 1], dt.float32, tag="outsb")

            for k in range(KC):
                nc.sync.dma_start(xtw[:, k, :], xtw_d[k])
            nc.sync.dma_start(negah[:], negah_d[:])
            nc.sync.dma_start(arows[:], arows_d[:])
            nc.sync.dma_start(ident[:], ident_d[:])
            nc.sync.dma_start(negbig[:], negbig_d[:])
            nc.sync.dma_start(ones1[:], ones1_d[:])
            nc.sync.dma_start(onescol[:], onescol_d[:])

            # Phase 1: matmuls + Sqrt into the dist buffer.
            last_sqrt = None
            for b in range(NBLK):
                for s in range(NSUB):
                    u = b * NSUB + s
                    row = 512 * b + 128 * s       # window col of this row-block
                    for g in range(NG):
                        ps = psump.tile([P, GRP], dt.float32, tag="ps")
                        base = 512 * b + g * GRP  # window col of group start
                        for t in range(TPG):
                            nc.tensor.matmul(
                                ps[:, t * 512:(t + 1) * 512],
                                ones1[:, :],
                                negah[:, base + t * 512: base + (t + 1) * 512],
                                start=True, stop=False,
                            )
                        if g == 0:
                            # Mask the true diagonal: psum += I.T @ (-BIG*I)
                            # so sq = -2*psum + a_i becomes huge -> exp ~ 0.
                            nc.tensor.matmul(
                                ps[:, 128 * s: 128 * (s + 1)],
                                ident[:, :], negbig[:, :],
                                start=False, stop=False,
                            )
                        for k in range(KC):
                            for t in range(TPG):
                                nc.tensor.matmul(
                                    ps[:, t * 512:(t + 1) * 512],
                                    xtw[:, k, row: row + 128],
                                    xtw[:, k, base + t * 512: base + (t + 1) * 512],
                                    start=False, stop=(k == KC - 1),
                                )
                        doff = u * BAND + g * GRP
                        last_sqrt = nc.scalar.activation(
                            dist[:, doff: doff + GRP],
                            ps[:, :],
                            AF.Sqrt,
                            bias=arows[:, u: u + 1],
                            scale=-2.0,
                        )

            # Phase 2: Exp with per-partition accumulation.
            # acc columns: [0:8] = D0 per sub-block, [8:16] = D8, [16:24] = mid.
            for u in range(NU):
                base = u * BAND
                segs = [
                    (0, 512, u),              # delta = 0 (diagonal block)
                    (512, BAND - 512, 16 + u),  # middle (counted twice)
                    (BAND - 512, BAND, 8 + u),  # delta = 8
                ]
                for lo, hi, col in segs:
                    e = nc.scalar.activation(
                        dist[:, base + lo: base + hi],
                        dist[:, base + lo: base + hi],
                        AF.Exp,
                        scale=-0.1,
                        accum_out=acc[:, col: col + 1],
                    )
                    # Keep all Exps after all Sqrts in ACT order: exactly one
                    # table switch instead of ping-ponging table loads.
                    add_dep_helper(e.ins, last_sqrt.ins, sync=False,
                                   reason="act table phase")

            # Epilogue: reduce accumulator columns, then across partitions.
            nc.vector.tensor_reduce(
                red2[:, 0:1], acc[:, 0:16], axis=mybir.AxisListType.X,
                op=mybir.AluOpType.add,
            )
            nc.vector.tensor_reduce(
                red2[:, 1:2], acc[:, 16:24], axis=mybir.AxisListType.X,
                op=mybir.AluOpType.add,
            )
            ps2 = psump1.tile([2, 1], dt.float32, tag="ps2")
            nc.tensor.matmul(ps2[:, :], red2[:, :], onescol[:, :],
                             start=True, stop=True)
            nc.vector.tensor_copy(outsb[:], ps2[:])
            nc.sync.dma_start(out_d[:], outsb[:])

    nc.finalize()
    return nc


def prepare_inputs(x):
    """Host-side sharding: per-core input dicts for run_bass_kernel_spmd."""
    x = np.ascontiguousarray(np.asarray(x, dtype=np.float32).reshape(N, D))
    a = (x.astype(np.float64) ** 2).sum(axis=1)          # true row norms
    xq = x.astype(np.float16)
    xT = np.ascontiguousarray(xq.T)                       # [512, 8192]

    ident = np.eye(P, dtype=np.float16)
    negbig = (-BIGVAL * np.eye(P)).astype(np.float16)
    ones1 = np.ones((1, P), dtype=np.float16)
    onescol = np.ones((P, 1), dtype=np.float32)

    in_maps = []
    for c in range(NCORES):
        win = (1024 * c + np.arange(WIN)) % N             # window col -> row
        xtw = np.ascontiguousarray(
            xT[:, win].reshape(KC, P, WIN))               # [4, 128, 5120]
        negah = np.ascontiguousarray(
            (-(a[win]) / 2.0).astype(np.float16).reshape(1, WIN))
        rows = 1024 * c + np.arange(1024)
        arows = np.ascontiguousarray(
            a[rows].astype(np.float32).reshape(NU, P).T)  # [128, 8]
        in_maps.append({
            "xtw": xtw,
            "negah": negah,
            "arows": arows,
            "ident": ident,
            "negbig": negbig,
            "ones1": ones1,
            "onescol": onescol,
        })
    return in_maps


def combine_outputs(results):
    """Combine per-core [2,1] outputs into the final loss values."""
    S = 0.0
    for r in results:
        o = np.asarray(r["out2"], dtype=np.float64).reshape(2)
        S += o[0] + 2.0 * o[1]
    S += float(N)  # exact diagonal contribution (masked to 0 on device)
    loss = 0.1 * S / (float(N) * float(N))
    return np.float32(loss), np.float32(0.5 * loss)


_CACHE = {}


def _get_program():
    if "nc" not in _CACHE:
        _CACHE["nc"] = build_program()
    return _CACHE["nc"]


def run(embeddings, trace=False):
    """Run the Bass kernel on 8 cores; returns (loss, total, BassKernelResults)."""
    nc = _get_program()
    in_maps = prepare_inputs(embeddings)
    res = run_bass_kernel_spmd(nc, in_maps, core_ids=list(range(NCORES)),
                               trace=trace)
    loss, total = combine_outputs(res.results)
    return loss, total, res


def kernel(embeddings):
    loss, total, _ = run(embeddings, trace=False)
    return loss, total


# revision 14
# speedup vs baseline: 1.3152x; 1.3152x over previous
"""Trainium2 Bass kernel for nn_ChallengingGeometricLoss.

Computes loss = 0.1 * mean(exp(-0.1 * cdist(x, x)))  for x = embeddings
reshaped to [N=8192, d=512], plus total = 0.5 * loss.

Strategy (8 NeuronCores, SPMD, identical program per core):
  - Rows are grouped in 16 super-blocks of 512. Super-block B computes its
    [512, 4608] cyclic band of the pairwise matrix: columns
    [512*B, 512*B + 4608) mod 8192 (its own diagonal block plus the next
    8 blocks).  With acc_all = sum over a band of exp(-0.1*dist) and
    accD = the delta=0 and delta=8 edge tiles:
        S = 2*sum(acc_all) - sum(accD) + N
    (middle deltas are counted twice by symmetry; edges once; the true
    diagonal is masked to ~0 on device and the exact +N added on host).
  - Core c owns super-blocks {2c, 2c+1}; all the columns it needs form a
    contiguous (mod N) window of 5120 rows, prepared host-side per core.
  - Per [128, 512] psum tile: a K=1 fp16 matmul broadcasts -a_j/2 into
    PSUM (start=True), then fp8e4m3 DoubleRow matmuls (K=2x128 each)
    accumulate x @ x.T.  The true-diagonal 128-col block also gets
    I.T @ (-BIG*I) added, masking it.  ACT computes
    dist = Sqrt(-2*psum + a_i) (bias = per-partition row norms) into a
    big fp16 SBUF buffer; after ALL sqrts (one activation table), a
    second ACT pass computes Exp(-0.1*dist) in place with accum_out
    (one table switch total). DVE re-reduces the delta-0/8 edge columns
    of the exponentials for the single-counted correction.
  - DVE reduces the accumulator columns, a ones-matmul reduces across
    partitions, and each core DMAs out two scalars [d_sum, t_sum].
    Host: S = sum_c (2*t_c - d_c) + N;  loss = 0.1 * S / N^2.
"""

import ml_dtypes
import numpy as np

import concourse.bass as bass
import concourse.mybir as mybir
import concourse.tile as tile
from concourse import bacc
from concourse.bass_utils import run_bass_kernel_spmd
from concourse.tile import add_dep_helper

# Problem constants (hardcoded per contract).
N = 8192
D = 512
NCORES = 8
P = 128
KC = D // P            # 4 k-chunks of 128
NBLK = 2               # local 512-row super-blocks per core
NSUB = 4               # 128-row sub-blocks per super-block
BAND = 4608            # band width per sub-block (9 x 512)
WIN = 5120             # per-core column window (2*512 + 4608)
GRP = 1536             # psum group: 3 banks = 3 x 512 cols
NG = BAND // GRP       # 3 psum groups per band
TPG = GRP // 512       # 3 x 512-col tiles per group
NU = NBLK * NSUB       # 8 sub-blocks per core
BIGVAL = 60000.0       # diagonal mask magnitude (exact in fp16)

MAIN_FP8 = True        # fp8e4m3 + DoubleRow mains vs fp16 mains

dt = mybir.dt
AF = mybir.ActivationFunctionType


def build_program(main_fp8=MAIN_FP8):
    """Build the per-core Bass/Tile program (identical across cores)."""
    nc = bacc.Bacc("TRN2", num_devices=NCORES, debug=False)

    main_dt = dt.float8e4 if main_fp8 else dt.float16
    xtw_d = nc.dram_tensor("xtw", [KC, P, WIN], main_dt, kind="ExternalInput")
    negah_d = nc.dram_tensor("negah", [1, WIN], dt.float16, kind="ExternalInput")
    arows_d = nc.dram_tensor("arows", [P, NU], dt.float32, kind="ExternalInput")
    ident_d = nc.dram_tensor("ident", [P, P], dt.float16, kind="ExternalInput")
    negbig_d = nc.dram_tensor("negbig", [P, P], dt.float16, kind="ExternalInput")
    ones1_d = nc.dram_tensor("ones1", [1, P], dt.float16, kind="ExternalInput")
    onescol_d = nc.dram_tensor("onescol", [P, 1], dt.float32, kind="ExternalInput")
    out_d = nc.dram_tensor("out2", [2, 1], dt.float32, kind="ExternalOutput")

    with tile.TileContext(nc) as tc:
        with (
            tc.tile_pool(name="big", bufs=1) as bigp,
            tc.tile_pool(name="small", bufs=1) as smallp,
            tc.tile_pool(name="psum", bufs=2, space="PSUM") as psump,
            tc.tile_pool(name="psum1", bufs=1, space="PSUM") as psump1,
        ):
            xtw = bigp.tile([P, KC, WIN], main_dt, tag="xtw")
            dist = bigp.tile([P, NU * BAND], dt.float16, tag="dist")
            negah = smallp.tile([1, WIN], dt.float16, tag="negah")
            arows = smallp.tile([P, NU], dt.float32, tag="arows")
            ident = smallp.tile([P, P], dt.float16, tag="ident")
            negbig = smallp.tile([P, P], dt.float16, tag="negbig")
            ones1 = smallp.tile([1, P], dt.float16, tag="ones1")
            onescol = smallp.tile([P, 1], dt.float32, tag="onescol")
            acc = smallp.tile([P, 3 * NU], dt.float32, tag="acc")
            red2 = smallp.tile([P, 2], dt.float32, tag="red2")
            outsb = smallp.tile([2, 1], dt.float32, tag="outsb")

            # Split each k-chunk DMA into halves so early matmuls can start
            # before the whole window has landed.
            H = WIN // 2
            for k in range(KC):
                nc.sync.dma_start(xtw[:, k, 0:H], xtw_d[k, :, 0:H])
                nc.sync.dma_start(xtw[:, k, H:WIN], xtw_d[k, :, H:WIN])
            nc.sync.dma_start(negah[:], negah_d[:])
            nc.sync.dma_start(arows[:], arows_d[:])
            nc.sync.dma_start(ident[:], ident_d[:])
            nc.sync.dma_start(negbig[:], negbig_d[:])
            nc.sync.dma_start(ones1[:], ones1_d[:])
            nc.sync.dma_start(onescol[:], onescol_d[:])

            # Phase 1: matmuls + Sqrt into the dist buffer.
            last_sqrt = None
            for b in range(NBLK):
                for s in range(NSUB):
                    u = b * NSUB + s
                    row = 512 * b + 128 * s       # window col of this row-block
                    for g in range(NG):
                        ps = psump.tile([P, GRP], dt.float32, tag="ps")
                        base = 512 * b + g * GRP  # window col of group start
                        for t in range(TPG):
                            nc.tensor.matmul(
                                ps[:, t * 512:(t + 1) * 512],
                                ones1[:, :],
                                negah[:, base + t * 512: base + (t + 1) * 512],
                                start=True, stop=False,
                            )
                        if g == 0:
                            # Mask the true diagonal: psum += I.T @ (-BIG*I)
                            # so sq = -2*psum + a_i becomes huge -> exp ~ 0.
                            nc.tensor.matmul(
                                ps[:, 128 * s: 128 * (s + 1)],
                                ident[:, :], negbig[:, :],
                                start=False, stop=False,
                            )
                        if main_fp8:
                            for kp in range(KC // 2):
                                for t in range(TPG):
                                    nc.tensor.matmul(
                                        ps[:, t * 512:(t + 1) * 512],
                                        xtw[:, 2 * kp: 2 * kp + 2, row: row + 128],
                                        xtw[:, 2 * kp: 2 * kp + 2,
                                            base + t * 512: base + (t + 1) * 512],
                                        start=False, stop=(kp == KC // 2 - 1),
                                        perf_mode=mybir.MatmulPerfMode.DoubleRow,
                                    )
                        else:
                            for k in range(KC):
                                for t in range(TPG):
                                    nc.tensor.matmul(
                                        ps[:, t * 512:(t + 1) * 512],
                                        xtw[:, k, row: row + 128],
                                        xtw[:, k, base + t * 512: base + (t + 1) * 512],
                                        start=False, stop=(k == KC - 1),
                                    )
                        doff = u * BAND + g * GRP
                        last_sqrt = nc.scalar.activation(
                            dist[:, doff: doff + GRP],
                            ps[:, :],
                            AF.Sqrt,
                            bias=arows[:, u: u + 1],
                            scale=-2.0,
                        )

            # Phase 2: Exp in place with per-partition accumulation.
            # acc columns: [16:24] = acc_all per band; [0:8]/[8:16] = the
            # delta-0 / delta-8 edge sums, re-reduced on DVE from the
            # exponentials.
            for u in range(NU):
                base = u * BAND
                e = nc.scalar.activation(
                    dist[:, base: base + BAND],
                    dist[:, base: base + BAND],
                    AF.Exp,
                    scale=-0.1,
                    accum_out=acc[:, 16 + u: 17 + u],
                )
                add_dep_helper(e.ins, last_sqrt.ins, sync=False,
                               reason="act table phase")
                nc.vector.tensor_reduce(
                    acc[:, u: u + 1], dist[:, base: base + 512],
                    axis=mybir.AxisListType.X, op=mybir.AluOpType.add,
                )
                nc.vector.tensor_reduce(
                    acc[:, 8 + u: 9 + u], dist[:, base + BAND - 512: base + BAND],
                    axis=mybir.AxisListType.X, op=mybir.AluOpType.add,
                )

            # Epilogue: reduce accumulator columns, then across partitions.
            nc.vector.tensor_reduce(
                red2[:, 0:1], acc[:, 0:16], axis=mybir.AxisListType.X,
                op=mybir.AluOpType.add,
            )
            nc.vector.tensor_reduce(
                red2[:, 1:2], acc[:, 16:24], axis=mybir.AxisListType.X,
                op=mybir.AluOpType.add,
            )
            ps2 = psump1.tile([2, 1], dt.float32, tag="ps2")
            nc.tensor.matmul(ps2[:, :], red2[:, :], onescol[:, :],
                             start=True, stop=True)
            nc.vector.tensor_copy(outsb[:], ps2[:])
            nc.sync.dma_start(out_d[:], outsb[:])

    nc.finalize()
    return nc


def prepare_inputs(x, main_fp8=MAIN_FP8):
    """Host-side sharding: per-core input dicts for run_bass_kernel_spmd."""
    x = np.ascontiguousarray(np.asarray(x, dtype=np.float32).reshape(N, D))
    a = (x.astype(np.float64) ** 2).sum(axis=1)          # true row norms
    qdt = ml_dtypes.float8_e4m3 if main_fp8 else np.float16
    xq = x.astype(qdt)
    xT = np.ascontiguousarray(xq.T)                       # [512, 8192]

    ident = np.eye(P, dtype=np.float16)
    negbig = (-BIGVAL * np.eye(P)).astype(np.float16)
    ones1 = np.ones((1, P), dtype=np.float16)
    onescol = np.ones((P, 1), dtype=np.float32)

    in_maps = []
    for c in range(NCORES):
        win = (1024 * c + np.arange(WIN)) % N             # window col -> row
        xtw = np.ascontiguousarray(
            xT[:, win].reshape(KC, P, WIN))               # [4, 128, 5120]
        negah = np.ascontiguousarray(
            (-(a[win]) / 2.0).astype(np.float16).reshape(1, WIN))
        rows = 1024 * c + np.arange(1024)
        arows = np.ascontiguousarray(
            a[rows].astype(np.float32).reshape(NU, P).T)  # [128, 8]
        in_maps.append({
            "xtw": xtw,
            "negah": negah,
            "arows": arows,
            "ident": ident,
            "negbig": negbig,
            "ones1": ones1,
            "onescol": onescol,
        })
    return in_maps


def combine_outputs(results):
    """Combine per-core [2,1] outputs into the final loss values."""
    S = 0.0
    for r in results:
        o = np.asarray(r["out2"], dtype=np.float64).reshape(2)
        S += 2.0 * o[1] - o[0]
    S += float(N)  # exact diagonal contribution (masked to 0 on device)
    loss = 0.1 * S / (float(N) * float(N))
    return np.float32(loss), np.float32(0.5 * loss)


_CACHE = {}


def _get_program():
    if "nc" not in _CACHE:
        _CACHE["nc"] = build_program()
    return _CACHE["nc"]


def run(embeddings, trace=False):
    """Run the Bass kernel on 8 cores; returns (loss, total, BassKernelResults)."""
    nc = _get_program()
    in_maps = prepare_inputs(embeddings)
    res = run_bass_kernel_spmd(nc, in_maps, core_ids=list(range(NCORES)),
                               trace=trace)
    loss, total = combine_outputs(res.results)
    return loss, total, res


def kernel(embeddings):
    loss, total, _ = run(embeddings, trace=False)
    return loss, total


# revision 16
# speedup vs baseline: 1.3837x; 1.0521x over previous
"""Trainium2 Bass kernel for nn_ChallengingGeometricLoss.

Computes loss = 0.1 * mean(exp(-0.1 * cdist(x, x)))  for x = embeddings
reshaped to [N=8192, d=512], plus total = 0.5 * loss.

Strategy (8 NeuronCores, SPMD, identical program per core):
  - Rows are grouped in 16 super-blocks of 512. Super-block B computes its
    [512, 4608] cyclic band of the pairwise matrix: columns
    [512*B, 512*B + 4608) mod 8192 (its own diagonal block plus the next
    8 blocks).  With acc_all = sum over a band of exp(-0.1*dist) and
    accD = the delta=0 and delta=8 edge tiles:
        S = 2*sum(acc_all) - sum(accD) + N
    (middle deltas are counted twice by symmetry; edges once; the true
    diagonal is masked to ~0 on device and the exact +N added on host).
  - Core c owns super-blocks {2c, 2c+1}; all the columns it needs form a
    contiguous (mod N) window of 5120 rows, prepared host-side per core.
  - Per [128, 512] psum tile: a K=1 fp16 matmul broadcasts -a_j/2 into
    PSUM (start=True), then fp8e4m3 DoubleRow matmuls (K=2x128 each)
    accumulate x @ x.T.  The true-diagonal 128-col block also gets
    I.T @ (-BIG*I) added, masking it.  ACT computes
    dist = Sqrt(-2*psum + a_i) (bias = per-partition row norms) into a
    big fp16 SBUF buffer; after ALL sqrts (one activation table), a
    second ACT pass computes Exp(-0.1*dist) in place with accum_out
    (one table switch total). DVE re-reduces the delta-0/8 edge columns
    of the exponentials for the single-counted correction.
  - DVE reduces the accumulator columns, a ones-matmul reduces across
    partitions, and each core DMAs out two scalars [d_sum, t_sum].
    Host: S = sum_c (2*t_c - d_c) + N;  loss = 0.1 * S / N^2.
"""

import ml_dtypes
import numpy as np

import concourse.bass as bass
import concourse.mybir as mybir
import concourse.tile as tile
from concourse import bacc
from concourse.bass_utils import run_bass_kernel_spmd
from concourse.tile import add_dep_helper

# Problem constants (hardcoded per contract).
N = 8192
D = 512
NCORES = 8
P = 128
KC = D // P            # 4 k-chunks of 128
NBLK = 2               # local 512-row super-blocks per core
NSUB = 4               # 128-row sub-blocks per super-block
BAND = 4608            # band width per sub-block (9 x 512)
WIN = 5120             # per-core column window (2*512 + 4608)
GRP = 1536             # psum group: 3 banks = 3 x 512 cols
NG = BAND // GRP       # 3 psum groups per band
TPG = GRP // 512       # 3 x 512-col tiles per group
NU = NBLK * NSUB       # 8 sub-blocks per core
BIGVAL = 60000.0       # diagonal mask magnitude (exact in fp16)

MAIN_FP8 = True        # fp8e4m3 + DoubleRow mains vs fp16 mains

dt = mybir.dt
AF = mybir.ActivationFunctionType


def build_program(main_fp8=MAIN_FP8):
    """Build the per-core Bass/Tile program (identical across cores)."""
    nc = bacc.Bacc("TRN2", num_devices=NCORES, debug=False)

    main_dt = dt.float8e4 if main_fp8 else dt.float16
    xtw_d = nc.dram_tensor("xtw", [KC, P, WIN], main_dt, kind="ExternalInput")
    negah_d = nc.dram_tensor("negah", [1, WIN], dt.float16, kind="ExternalInput")
    arows_d = nc.dram_tensor("arows", [P, NU], dt.float32, kind="ExternalInput")
    ident_d = nc.dram_tensor("ident", [P, P], dt.float16, kind="ExternalInput")
    negbig_d = nc.dram_tensor("negbig", [P, P], dt.float16, kind="ExternalInput")
    ones1_d = nc.dram_tensor("ones1", [1, P], dt.float16, kind="ExternalInput")
    onescol_d = nc.dram_tensor("onescol", [P, 1], dt.float32, kind="ExternalInput")
    out_d = nc.dram_tensor("out2", [2, 1], dt.float32, kind="ExternalOutput")

    with tile.TileContext(nc) as tc:
        with (
            tc.tile_pool(name="big", bufs=1) as bigp,
            tc.tile_pool(name="small", bufs=1) as smallp,
            tc.tile_pool(name="psum", bufs=2, space="PSUM") as psump,
            tc.tile_pool(name="psum1", bufs=1, space="PSUM") as psump1,
        ):
            xtw = bigp.tile([P, KC, WIN], main_dt, tag="xtw")
            dist = bigp.tile([P, NU * BAND], dt.float16, tag="dist")
            negah = smallp.tile([1, WIN], dt.float16, tag="negah")
            arows = smallp.tile([P, NU], dt.float32, tag="arows")
            ident = smallp.tile([P, P], dt.float16, tag="ident")
            negbig = smallp.tile([P, P], dt.float16, tag="negbig")
            ones1 = smallp.tile([1, P], dt.float16, tag="ones1")
            onescol = smallp.tile([P, 1], dt.float32, tag="onescol")
            acc = smallp.tile([P, 3 * NU], dt.float32, tag="acc")
            red2 = smallp.tile([P, 2], dt.float32, tag="red2")
            outsb = smallp.tile([2, 1], dt.float32, tag="outsb")

            # Constants first (tiny, unblock warmup + the first matmuls),
            # spread across idle engines' queues to parallelize issue cost.
            nc.scalar.dma_start(ident[:], ident_d[:])
            nc.scalar.dma_start(negbig[:], negbig_d[:])
            nc.scalar.dma_start(ones1[:], ones1_d[:])
            nc.gpsimd.dma_start(negah[:], negah_d[:])
            nc.gpsimd.dma_start(arows[:], arows_d[:])
            nc.gpsimd.dma_start(onescol[:], onescol_d[:])
            # Split each k-chunk DMA into halves so early matmuls can start
            # before the whole window has landed.
            H = WIN // 2
            for k in range(KC):
                nc.sync.dma_start(xtw[:, k, 0:H], xtw_d[k, :, 0:H])
                nc.sync.dma_start(xtw[:, k, H:WIN], xtw_d[k, :, H:WIN])

            # PE warmup: keep TensorE busy through the DMA prologue so the
            # HAM clock gate opens (1.2 -> 2.4 GHz) before the real matmuls.
            warm = psump1.tile([P, P], dt.float32, tag="warm")
            for w in range(96):
                nc.tensor.matmul(warm[:, :], ident[:, :], negbig[:, :],
                                 start=True, stop=True)

            # Phase 1: matmuls + Sqrt into the dist buffer.
            last_sqrt = None
            for b in range(NBLK):
                for s in range(NSUB):
                    u = b * NSUB + s
                    row = 512 * b + 128 * s       # window col of this row-block
                    for g in range(NG):
                        ps = psump.tile([P, GRP], dt.float32, tag="ps")
                        base = 512 * b + g * GRP  # window col of group start
                        for t in range(TPG):
                            nc.tensor.matmul(
                                ps[:, t * 512:(t + 1) * 512],
                                ones1[:, :],
                                negah[:, base + t * 512: base + (t + 1) * 512],
                                start=True, stop=False,
                            )
                        if g == 0:
                            # Mask the true diagonal: psum += I.T @ (-BIG*I)
                            # so sq = -2*psum + a_i becomes huge -> exp ~ 0.
                            nc.tensor.matmul(
                                ps[:, 128 * s: 128 * (s + 1)],
                                ident[:, :], negbig[:, :],
                                start=False, stop=False,
                            )
                        if main_fp8:
                            for kp in range(KC // 2):
                                for t in range(TPG):
                                    nc.tensor.matmul(
                                        ps[:, t * 512:(t + 1) * 512],
                                        xtw[:, 2 * kp: 2 * kp + 2, row: row + 128],
                                        xtw[:, 2 * kp: 2 * kp + 2,
                                            base + t * 512: base + (t + 1) * 512],
                                        start=False, stop=(kp == KC // 2 - 1),
                                        perf_mode=mybir.MatmulPerfMode.DoubleRow,
                                    )
                        else:
                            for k in range(KC):
                                for t in range(TPG):
                                    nc.tensor.matmul(
                                        ps[:, t * 512:(t + 1) * 512],
                                        xtw[:, k, row: row + 128],
                                        xtw[:, k, base + t * 512: base + (t + 1) * 512],
                                        start=False, stop=(k == KC - 1),
                                    )
                        doff = u * BAND + g * GRP
                        last_sqrt = nc.scalar.activation(
                            dist[:, doff: doff + GRP],
                            ps[:, :],
                            AF.Sqrt,
                            bias=arows[:, u: u + 1],
                            scale=-2.0,
                        )

            # Phase 2: Exp in place with per-partition accumulation.
            # acc columns: [16:24] = acc_all per band; [0:8]/[8:16] = the
            # delta-0 / delta-8 edge sums, re-reduced on DVE from the
            # exponentials.
            for u in range(NU):
                base = u * BAND
                e = nc.scalar.activation(
                    dist[:, base: base + BAND],
                    dist[:, base: base + BAND],
                    AF.Exp,
                    scale=-0.1,
                    accum_out=acc[:, 16 + u: 17 + u],
                )
                add_dep_helper(e.ins, last_sqrt.ins, sync=False,
                               reason="act table phase")
                nc.vector.tensor_reduce(
                    acc[:, u: u + 1], dist[:, base: base + 512],
                    axis=mybir.AxisListType.X, op=mybir.AluOpType.add,
                )
                nc.vector.tensor_reduce(
                    acc[:, 8 + u: 9 + u], dist[:, base + BAND - 512: base + BAND],
                    axis=mybir.AxisListType.X, op=mybir.AluOpType.add,
                )

            # Epilogue: reduce accumulator columns, then across partitions.
            nc.vector.tensor_reduce(
                red2[:, 0:1], acc[:, 0:16], axis=mybir.AxisListType.X,
                op=mybir.AluOpType.add,
            )
            nc.vector.tensor_reduce(
                red2[:, 1:2], acc[:, 16:24], axis=mybir.AxisListType.X,
                op=mybir.AluOpType.add,
            )
            ps2 = psump1.tile([2, 1], dt.float32, tag="ps2")
            nc.tensor.matmul(ps2[:, :], red2[:, :], onescol[:, :],
                             start=True, stop=True)
            nc.vector.tensor_copy(outsb[:], ps2[:])
            nc.sync.dma_start(out_d[:], outsb[:])

    nc.finalize()
    return nc


def prepare_inputs(x, main_fp8=MAIN_FP8):
    """Host-side sharding: per-core input dicts for run_bass_kernel_spmd."""
    x = np.ascontiguousarray(np.asarray(x, dtype=np.float32).reshape(N, D))
    a = (x.astype(np.float64) ** 2).sum(axis=1)          # true row norms
    qdt = ml_dtypes.float8_e4m3 if main_fp8 else np.float16
    xq = x.astype(qdt)
    xT = np.ascontiguousarray(xq.T)                       # [512, 8192]

    ident = np.eye(P, dtype=np.float16)
    negbig = (-BIGVAL * np.eye(P)).astype(np.float16)
    ones1 = np.ones((1, P), dtype=np.float16)
    onescol = np.ones((P, 1), dtype=np.float32)

    in_maps = []
    for c in range(NCORES):
        win = (1024 * c + np.arange(WIN)) % N             # window col -> row
        xtw = np.ascontiguousarray(
            xT[:, win].reshape(KC, P, WIN))               # [4, 128, 5120]
        negah = np.ascontiguousarray(
            (-(a[win]) / 2.0).astype(np.float16).reshape(1, WIN))
        rows = 1024 * c + np.arange(1024)
        arows = np.ascontiguousarray(
            a[rows].astype(np.float32).reshape(NU, P).T)  # [128, 8]
        in_maps.append({
            "xtw": xtw,
            "negah": negah,
            "arows": arows,
            "ident": ident,
            "negbig": negbig,
            "ones1": ones1,
            "onescol": onescol,
        })
    return in_maps


def combine_outputs(results):
    """Combine per-core [2,1] outputs into the final loss values."""
    S = 0.0
    for r in results:
        o = np.asarray(r["out2"], dtype=np.float64).reshape(2)
        S += 2.0 * o[1] - o[0]
    S += float(N)  # exact diagonal contribution (masked to 0 on device)
    loss = 0.1 * S / (float(N) * float(N))
    return np.float32(loss), np.float32(0.5 * loss)


_CACHE = {}


def _get_program():
    if "nc" not in _CACHE:
        _CACHE["nc"] = build_program()
    return _CACHE["nc"]


def run(embeddings, trace=False):
    """Run the Bass kernel on 8 cores; returns (loss, total, BassKernelResults)."""
    nc = _get_program()
    in_maps = prepare_inputs(embeddings)
    res = run_bass_kernel_spmd(nc, in_maps, core_ids=list(range(NCORES)),
                               trace=trace)
    loss, total = combine_outputs(res.results)
    return loss, total, res


def kernel(embeddings):
    loss, total, _ = run(embeddings, trace=False)
    return loss, total


# revision 24
# speedup vs baseline: 1.4222x; 1.0278x over previous
"""Trainium2 Bass kernel for nn_ChallengingGeometricLoss.

Computes loss = 0.1 * mean(exp(-0.1 * cdist(x, x)))  for x = embeddings
reshaped to [N=8192, d=512], plus total = 0.5 * loss.

Strategy (8 NeuronCores, SPMD, identical program per core):
  - Rows are grouped in 16 super-blocks of 512. Super-block B computes its
    [512, 4608] cyclic band of the pairwise matrix: columns
    [512*B, 512*B + 4608) mod 8192 (its own diagonal block plus the next
    8 blocks).  With acc_all = sum over a band of exp(-0.1*dist) and
    accD = the delta=0 and delta=8 edge tiles:
        S = 2*sum(acc_all) - sum(accD) + N
    (middle deltas are counted twice by symmetry; edges once; the true
    diagonal is masked to ~0 on device and the exact +N added on host).
  - Core c owns super-blocks {2c, 2c+1}; all the columns it needs form a
    contiguous (mod N) window of 5120 rows, prepared host-side per core.
  - Per [128, 512] psum tile: a K=1 fp16 matmul broadcasts -a_j/2 into
    PSUM (start=True), then fp8e4m3 DoubleRow matmuls (K=2x128 each)
    accumulate x @ x.T.  The true-diagonal 128-col block also gets
    I.T @ (-BIG*I) added, masking it.  ACT computes
    dist = Sqrt(-2*psum + a_i) (bias = per-partition row norms) into a
    big fp16 SBUF buffer; after ALL sqrts (one activation table), a
    second ACT pass computes Exp(-0.1*dist) in place with accum_out
    (one table switch total). DVE re-reduces the delta-0/8 edge columns
    of the exponentials for the single-counted correction.
  - DVE reduces the accumulator columns, a ones-matmul reduces across
    partitions, and each core DMAs out two scalars [d_sum, t_sum].
    Host: S = sum_c (2*t_c - d_c) + N;  loss = 0.1 * S / N^2.
"""

import ml_dtypes
import numpy as np

import concourse.bass as bass
import concourse.mybir as mybir
import concourse.tile as tile
from concourse import bacc
from concourse.bass_utils import run_bass_kernel_spmd
from concourse.tile import add_dep_helper

# Problem constants (hardcoded per contract).
N = 8192
D = 512
NCORES = 8
P = 128
KC = D // P            # 4 k-chunks of 128
NBLK = 2               # local 512-row super-blocks per core
NSUB = 4               # 128-row sub-blocks per super-block
BAND = 4608            # band width per sub-block (9 x 512)
WIN = 5120             # per-core column window (2*512 + 4608)
GRP = 1536             # psum group: 3 banks = 3 x 512 cols
NG = BAND // GRP       # 3 psum groups per band
TPG = GRP // 512       # 3 x 512-col tiles per group
NU = NBLK * NSUB       # 8 sub-blocks per core
BIGVAL = 60000.0       # diagonal mask magnitude (exact in fp16)

MAIN_FP8 = True        # fp8e4m3 + DoubleRow mains vs fp16 mains

dt = mybir.dt
AF = mybir.ActivationFunctionType


def build_program(main_fp8=MAIN_FP8):
    """Build the per-core Bass/Tile program (identical across cores)."""
    nc = bacc.Bacc("TRN2", num_devices=NCORES, debug=False)

    main_dt = dt.float8e4 if main_fp8 else dt.float16
    xtw_d = nc.dram_tensor("xtw", [KC, P, WIN], main_dt, kind="ExternalInput")
    negah_d = nc.dram_tensor("negah", [1, WIN], dt.float16, kind="ExternalInput")
    apos_d = nc.dram_tensor("apos", [1, WIN], dt.float16, kind="ExternalInput")
    arows_d = nc.dram_tensor("arows", [P, NU], dt.float32, kind="ExternalInput")
    ident_d = nc.dram_tensor("ident", [P, P], dt.float16, kind="ExternalInput")
    negbig_d = nc.dram_tensor("negbig", [P, P], dt.float16, kind="ExternalInput")
    ones1_d = nc.dram_tensor("ones1", [1, P], dt.float16, kind="ExternalInput")
    onescol_d = nc.dram_tensor("onescol", [P, 1], dt.float32, kind="ExternalInput")
    out_d = nc.dram_tensor("out2", [2, 1], dt.float32, kind="ExternalOutput")

    with tile.TileContext(nc) as tc:
        with (
            tc.tile_pool(name="big", bufs=1) as bigp,
            tc.tile_pool(name="small", bufs=1) as smallp,
            tc.tile_pool(name="psum", bufs=2, space="PSUM") as psump,
            tc.tile_pool(name="psum1", bufs=1, space="PSUM") as psump1,
        ):
            xtw = bigp.tile([P, KC, WIN], main_dt, tag="xtw")
            dist = bigp.tile([P, NU * BAND], dt.float16, tag="dist")
            a2b = bigp.tile([P, WIN], dt.float16, tag="a2b")
            negah = smallp.tile([1, WIN], dt.float16, tag="negah")
            arows = smallp.tile([P, NU], dt.float32, tag="arows")
            ident = smallp.tile([P, P], dt.float16, tag="ident")
            negbig = smallp.tile([P, P], dt.float16, tag="negbig")
            ones1 = smallp.tile([1, P], dt.float16, tag="ones1")
            onescol = smallp.tile([P, 1], dt.float32, tag="onescol")
            acc = smallp.tile([P, 3 * NU], dt.float32, tag="acc")
            red2 = smallp.tile([P, 2], dt.float32, tag="red2")
            outsb = smallp.tile([2, 1], dt.float32, tag="outsb")

            # Constants first (tiny, unblock warmup + the first matmuls),
            # spread across idle engines' queues to parallelize issue cost.
            nc.scalar.dma_start(ident[:], ident_d[:])
            nc.scalar.dma_start(negbig[:], negbig_d[:])
            nc.scalar.dma_start(ones1[:], ones1_d[:])
            nc.gpsimd.dma_start(negah[:], negah_d[:])
            nc.gpsimd.dma_start(arows[:], arows_d[:])
            nc.gpsimd.dma_start(onescol[:], onescol_d[:])
            # Broadcast the positive row-norm row across all 128 partitions.
            apos_src = apos_d[:]
            apos_bcast = bass.AP(
                tensor=apos_src.tensor,
                offset=apos_src.offset,
                ap=[[0, P], apos_src.ap[1]],
            )
            nc.gpsimd.dma_start(a2b[:, :], apos_bcast)
            # Split each k-chunk DMA into halves so early matmuls can start
            # before the whole window has landed.
            H = WIN // 2
            for k in range(KC):
                nc.sync.dma_start(xtw[:, k, 0:H], xtw_d[k, :, 0:H])
                nc.sync.dma_start(xtw[:, k, H:WIN], xtw_d[k, :, H:WIN])

            # PE warmup: keep TensorE busy through the DMA prologue so the
            # HAM clock gate opens (1.2 -> 2.4 GHz) before the real matmuls.
            warm = psump1.tile([P, P], dt.float32, tag="warm")
            for w in range(72):
                nc.tensor.matmul(warm[:, :], ident[:, :], negbig[:, :],
                                 start=True, stop=True)

            # Phase 1: matmuls + Sqrt into the dist buffer.
            last_sqrt = None
            for b in range(NBLK):
                for s in range(NSUB):
                    u = b * NSUB + s
                    row = 512 * b + 128 * s       # window col of this row-block
                    for g in range(NG):
                        ps = psump.tile([P, GRP], dt.float32, tag="ps")
                        base = 512 * b + g * GRP  # window col of group start
                        pe_aug = (g == 0)
                        if pe_aug:
                            # -a_j/2 broadcast into psum via a K=1 matmul.
                            for t in range(TPG):
                                nc.tensor.matmul(
                                    ps[:, t * 512:(t + 1) * 512],
                                    ones1[:, :],
                                    negah[:, base + t * 512: base + (t + 1) * 512],
                                    start=True, stop=False,
                                )
                            # Mask the true diagonal: psum += I.T @ (-BIG*I)
                            # so sq = -2*psum + a_i becomes huge -> exp ~ 0.
                            nc.tensor.matmul(
                                ps[:, 128 * s: 128 * (s + 1)],
                                ident[:, :], negbig[:, :],
                                start=False, stop=False,
                            )
                        if main_fp8:
                            for kp in range(KC // 2):
                                for t in range(TPG):
                                    nc.tensor.matmul(
                                        ps[:, t * 512:(t + 1) * 512],
                                        xtw[:, 2 * kp: 2 * kp + 2, row: row + 128],
                                        xtw[:, 2 * kp: 2 * kp + 2,
                                            base + t * 512: base + (t + 1) * 512],
                                        start=(not pe_aug and kp == 0),
                                        stop=(kp == KC // 2 - 1),
                                        perf_mode=mybir.MatmulPerfMode.DoubleRow,
                                    )
                        else:
                            for k in range(KC):
                                for t in range(TPG):
                                    nc.tensor.matmul(
                                        ps[:, t * 512:(t + 1) * 512],
                                        xtw[:, k, row: row + 128],
                                        xtw[:, k, base + t * 512: base + (t + 1) * 512],
                                        start=(not pe_aug and k == 0),
                                        stop=(k == KC - 1),
                                    )
                        doff = u * BAND + g * GRP
                        if pe_aug:
                            # dist = sqrt(-2*psum + a_i); psum = dot - a_j/2.
                            last_sqrt = nc.scalar.activation(
                                dist[:, doff: doff + GRP],
                                ps[:, :],
                                AF.Sqrt,
                                bias=arows[:, u: u + 1],
                                scale=-2.0,
                            )
                        else:
                            # Offload the a_j add to the (otherwise idle) DVE:
                            # sq32 = -2*psum + a_j, then dist = sqrt(sq32+a_i).
                            sq32 = bigp.tile([P, GRP], dt.float32, tag="sq32",
                                             bufs=3)
                            nc.vector.scalar_tensor_tensor(
                                sq32[:, :], ps[:, :], -2.0,
                                a2b[:, base: base + GRP],
                                op0=mybir.AluOpType.mult,
                                op1=mybir.AluOpType.add,
                            )
                            last_sqrt = nc.scalar.activation(
                                dist[:, doff: doff + GRP],
                                sq32[:, :],
                                AF.Sqrt,
                                bias=arows[:, u: u + 1],
                                scale=1.0,
                            )

            # Phase 2: Exp in place with per-partition accumulation.
            # acc columns: [16:24] = acc_all per band; [0:8]/[8:16] = the
            # delta-0 / delta-8 edge sums, re-reduced on DVE from the
            # exponentials.
            for u in range(NU):
                base = u * BAND
                e = nc.scalar.activation(
                    dist[:, base: base + BAND],
                    dist[:, base: base + BAND],
                    AF.Exp,
                    scale=-0.1,
                    accum_out=acc[:, 16 + u: 17 + u],
                )
                add_dep_helper(e.ins, last_sqrt.ins, sync=False,
                               reason="act table phase")
                nc.vector.tensor_reduce(
                    acc[:, u: u + 1], dist[:, base: base + 512],
                    axis=mybir.AxisListType.X, op=mybir.AluOpType.add,
                )
                nc.vector.tensor_reduce(
                    acc[:, 8 + u: 9 + u], dist[:, base + BAND - 512: base + BAND],
                    axis=mybir.AxisListType.X, op=mybir.AluOpType.add,
                )

            # Epilogue: reduce accumulator columns, then across partitions.
            nc.vector.tensor_reduce(
                red2[:, 0:1], acc[:, 0:16], axis=mybir.AxisListType.X,
                op=mybir.AluOpType.add,
            )
            nc.vector.tensor_reduce(
                red2[:, 1:2], acc[:, 16:24], axis=mybir.AxisListType.X,
                op=mybir.AluOpType.add,
            )
            ps2 = psump1.tile([2, 1], dt.float32, tag="ps2")
            nc.tensor.matmul(ps2[:, :], red2[:, :], onescol[:, :],
                             start=True, stop=True)
            nc.vector.tensor_copy(outsb[:], ps2[:])
            nc.sync.dma_start(out_d[:], outsb[:])

    nc.finalize()
    return nc


def prepare_inputs(x, main_fp8=MAIN_FP8):
    """Host-side sharding: per-core input dicts for run_bass_kernel_spmd."""
    x = np.ascontiguousarray(np.asarray(x, dtype=np.float32).reshape(N, D))
    a = (x.astype(np.float64) ** 2).sum(axis=1)          # true row norms
    qdt = ml_dtypes.float8_e4m3 if main_fp8 else np.float16
    xq = x.astype(qdt)
    xT = np.ascontiguousarray(xq.T)                       # [512, 8192]

    ident = np.eye(P, dtype=np.float16)
    negbig = (-BIGVAL * np.eye(P)).astype(np.float16)
    ones1 = np.ones((1, P), dtype=np.float16)
    onescol = np.ones((P, 1), dtype=np.float32)

    in_maps = []
    for c in range(NCORES):
        win = (1024 * c + np.arange(WIN)) % N             # window col -> row
        xtw = np.ascontiguousarray(
            xT[:, win].reshape(KC, P, WIN))               # [4, 128, 5120]
        negah = np.ascontiguousarray(
            (-(a[win]) / 2.0).astype(np.float16).reshape(1, WIN))
        apos = np.ascontiguousarray(a[win].astype(np.float16).reshape(1, WIN))
        rows = 1024 * c + np.arange(1024)
        arows = np.ascontiguousarray(
            a[rows].astype(np.float32).reshape(NU, P).T)  # [128, 8]
        in_maps.append({
            "xtw": xtw,
            "negah": negah,
            "apos": apos,
            "arows": arows,
            "ident": ident,
            "negbig": negbig,
            "ones1": ones1,
            "onescol": onescol,
        })
    return in_maps


def combine_outputs(results):
    """Combine per-core [2,1] outputs into the final loss values."""
    S = 0.0
    for r in results:
        o = np.asarray(r["out2"], dtype=np.float64).reshape(2)
        S += 2.0 * o[1] - o[0]
    S += float(N)  # exact diagonal contribution (masked to 0 on device)
    loss = 0.1 * S / (float(N) * float(N))
    return np.float32(loss), np.float32(0.5 * loss)


_CACHE = {}


def _get_program():
    if "nc" not in _CACHE:
        _CACHE["nc"] = build_program()
    return _CACHE["nc"]


def run(embeddings, trace=False):
    """Run the Bass kernel on 8 cores; returns (loss, total, BassKernelResults)."""
    nc = _get_program()
    in_maps = prepare_inputs(embeddings)
    res = run_bass_kernel_spmd(nc, in_maps, core_ids=list(range(NCORES)),
                               trace=trace)
    loss, total = combine_outputs(res.results)
    return loss, total, res


def kernel(embeddings):
    loss, total, _ = run(embeddings, trace=False)
    return loss, total
